# revision 8
# baseline (speedup 1.0000x reference)
"""Causal single-head attention (Q==K source bug faithful) on 8 TRN2 NeuronCores.

Problem: x [4, 4096, 1024], Wk/Wv [1024, 64];
  k = q = x@Wk; scores = q k^T / 8, causal softmax, out = weights @ (x@Wv).

Strategy (no collectives, uniform SPMD program):
  - 8 cores = 4 batches x 2 roles. Each core computes the full K/V
    projection for its batch (redundantly) and owns 2 query chunks of
    1024 rows: role A owns global chunks {0, 3}, role B owns {1, 2}.
    This balances causal attention work exactly (pairs: 1+7 == 3+5).
  - One compiled program for all cores. Per-core differences are pure
    data: the host permutes the 8 key panels (512 rows each) so that
    panels 0,1 = "lo" chunk rows, 2,3 = "hi" chunk rows, 4..7 = the
    rest in global order. Causal key validity is data too: a 0/1
    "vmask" per (chunk, key-block) zeroes the ones-augmented V columns
    of keys that a chunk must not see, so template padding slots
    contribute exactly zero to both numerator and denominator
    (softmax without max-subtraction is safe here: scores <= ~16).
  - Flash-style transposed layout: scores^T [keys_p, queries_f] via
    TensorE, exp on ScalarE (scale=1/8 fused), P@V via TensorE with
    ones-row V giving the denominator for free, PSUM accumulation
    across key blocks, PE-transpose epilogue + per-partition
    reciprocal multiply on VectorE.
  - v3 perf: bf16 host-fed inputs (halves HBM traffic); all panel
    DMAs issued upfront on two queues; software-pipelined slots (the
    scores matmuls of slot i+1 issue before the P@V matmuls of slot
    i) so TensorE never stalls on ScalarE's exp and holds max
    p-state; chunk-serial schedule with a single live PSUM
    accumulator; separate PSUM pool for projection/epilogue tiles so
    they don't serialize against the scores-tile rotation; the lo
    epilogue is interleaved piecewise between the following slots.
"""
import numpy as np
import ml_dtypes

import concourse.bass as bass
import concourse.mybir as mybir
from concourse import bacc, tile
from concourse.bass_utils import run_bass_kernel_spmd

F32 = mybir.dt.float32
BF16 = mybir.dt.bfloat16
EXP = mybir.ActivationFunctionType.Exp

B, T, C, H = 4, 4096, 1024, 64
NCHI = C // 128          # 8 contraction blocks
NPAN = 8                 # 512-row key panels per core
PAN = 512
NKB = 32                 # 128-row key blocks per core
CHUNK = 1024             # queries per chunk
# role -> (lo global chunk, hi global chunk)
ROLE_CHUNKS = {0: (0, 3), 1: (1, 2)}


def build_nc():
    nc = bacc.Bacc("TRN2", target_bir_lowering=False, debug=False, num_devices=8)

    xt_d = nc.declare_dram_parameter("xt", [NPAN, 128, NCHI, PAN], BF16, isOutput=False)
    wkv_d = nc.declare_dram_parameter("wkv", [128, NCHI, 128], BF16, isOutput=False)
    vm_d = nc.declare_dram_parameter("vm", [128, 2 * NKB], F32, isOutput=False)
    mk_d = nc.declare_dram_parameter("mk", [128, 1280], BF16, isOutput=False)
    eye_d = nc.declare_dram_parameter("eye", [128, 129], F32, isOutput=False)
    out_d = nc.declare_dram_parameter("out", [2 * CHUNK, H], F32, isOutput=True)
    out_v = out_d.ap().rearrange("(i p) h -> p i h", p=128)  # [128, 16, 64]

    with tile.TileContext(nc) as tc:
        with (
            tc.tile_pool(name="const", bufs=1) as const,
            tc.tile_pool(name="xt", bufs=NPAN) as xtp,
            tc.tile_pool(name="kv", bufs=2) as kvp,
            tc.tile_pool(name="pt", bufs=4) as ptp,
            tc.tile_pool(name="ot_sb", bufs=2) as otsbp,
            tc.tile_pool(name="rc", bufs=2) as rcp,
            tc.tile_pool(name="psA", bufs=3, space="PSUM") as psA,
            tc.tile_pool(name="psO", bufs=1, space="PSUM") as psO,
        ):
            wkv = const.tile([128, NCHI, 128], BF16, tag="wkv")
            vm = const.tile([128, 2 * NKB], F32, tag="vm")
            mk = const.tile([128, 1280], BF16, tag="mk")
            eye = const.tile([128, 129], F32, tag="eye")
            eyeb = const.tile([128, 64], BF16, tag="eyeb")
            kt = const.tile([64, T], BF16, tag="kt")          # K^T, permuted cols
            vaug = const.tile([128, NKB, 65], BF16, tag="vaug")    # lo-masked V|1
            vaug2 = const.tile([128, NKB, 65], BF16, tag="vaug2")  # hi-masked V|1
            outsb = const.tile([128, 16, H], F32, tag="outsb")

            nc.gpsimd.dma_start(wkv[:], wkv_d[:])
            nc.sync.dma_start(vm[:], vm_d[:])
            nc.sync.dma_start(mk[:], mk_d[:])
            nc.sync.dma_start(eye[:], eye_d[:])
            eye64 = eye[64:128, 0:64]
            eye65 = eye[0:65, 64:129]
            nc.vector.tensor_copy(eyeb[64:128, :], eye64)
            gmask = mk[:, 0:1024]
            bmask = mk[:, 1024:1280]

            vaugs = (vaug, vaug2)
            ot = [None, None]  # per-chunk PSUM accumulators [65, 1024]

            # All panel DMAs upfront, alternating queues.
            xts = []
            for p in range(NPAN):
                xt = xtp.tile([128, NCHI, PAN], BF16, tag="xt")
                (nc.gpsimd if p % 2 == 0 else nc.sync).dma_start(xt[:], xt_d[p])
                xts.append(xt)

            def proj_panel(p):
                """Project K^T/V^T for panel p, transpose V, build V|1."""
                xt = xts[p]
                kv_ps = psA.tile([128, PAN], F32, tag="ps")
                for ci in range(NCHI):
                    nc.tensor.matmul(
                        kv_ps[:], wkv[:, ci, :], xt[:, ci, :],
                        start=(ci == 0), stop=(ci == NCHI - 1),
                    )
                kvsb = kvp.tile([128, PAN], BF16, tag="kv")
                nc.vector.tensor_copy(kvsb[:], kv_ps[:])
                nc.vector.tensor_copy(kt[:, p * PAN:(p + 1) * PAN], kvsb[0:64, :])
                v_ps = psA.tile([128, 4, 64], BF16, tag="ps")
                for tb in range(4):
                    nc.tensor.transpose(
                        v_ps[:, tb, :], kvsb[64:128, tb * 128:(tb + 1) * 128],
                        eyeb[64:128, :],
                    )
                for which, vt in enumerate(vaugs):
                    vcol = vm[:, which * NKB + 4 * p: which * NKB + 4 * p + 4]
                    nc.vector.tensor_mul(
                        vt[:, 4 * p:4 * p + 4, 0:64],
                        v_ps[:],
                        vcol.unsqueeze(2).broadcast_to([128, 4, 64]),
                    )
                    nc.vector.tensor_copy(
                        vt[:, 4 * p:4 * p + 4, 64:65], vcol.unsqueeze(2)
                    )

            def qt(chunk):
                return kt[:, chunk * CHUNK:(chunk + 1) * CHUNK]

            def pieces(c0):
                """Split [c0, 1024) at the PSUM bank boundary (512 f32):
                one matmul must not write across banks."""
                if c0 >= 512:
                    return [(c0, 1024 - c0)]
                return [(c0, 512 - c0), (512, 512)]

            def scores_exp(chunk, kb, c0=0, mask=None):
                """Slot front half: scores^T -> exp -> (mask). Returns pt."""
                Q = qt(chunk)
                s_ps = psA.tile([128, 1024], F32, tag="ps")
                for (o, ln) in pieces(c0):
                    nc.tensor.matmul(
                        s_ps[:, o:o + ln],
                        kt[:, kb * 128:(kb + 1) * 128],
                        Q[:, o:o + ln],
                        start=True, stop=True,
                    )
                pt = ptp.tile([128, 1024], BF16, tag="pt")
                nc.scalar.activation(pt[:, c0:1024], s_ps[:, c0:1024], EXP, scale=0.125)
                if mask is not None:
                    nc.vector.tensor_mul(pt[:, c0:1024], pt[:, c0:1024], mask)
                return pt

            def pv(chunk, kb, pt, c0=0, first=False, last=False):
                """Slot back half: P@V accumulate into the chunk PSUM."""
                if first:
                    ot[chunk % 2] = psO.tile([65, 1024], F32, tag="ot", name="ot")
                acc = ot[chunk % 2]
                ps = pieces(c0)
                for n, (o, ln) in enumerate(ps):
                    nc.tensor.matmul(
                        acc[:, o:o + ln],
                        vaugs[chunk % 2][:, kb, :],
                        pt[:, o:o + ln],
                        start=first, stop=(last and n == len(ps) - 1),
                    )

            def epi_start(chunk):
                """Copy the finished accumulator to SBUF (frees PSUM)."""
                ci = chunk % 2
                osb = otsbp.tile([65, 1024], F32, tag="ot_sb")
                nc.vector.tensor_copy(osb[:], ot[ci][:])
                return osb

            def epi_piece(chunk, osb, i):
                """Transpose 128 rows back, divide by the denominator."""
                ci = chunk % 2
                tr = psA.tile([128, 65], F32, tag="ps")
                nc.tensor.transpose(tr[:], osb[:, i * 128:(i + 1) * 128], eye65)
                rc = rcp.tile([128, 1], F32, tag="rc")
                nc.vector.reciprocal(rc[:], tr[:, 64:65])
                nc.vector.tensor_scalar_mul(
                    outsb[:, 8 * ci + i, :], tr[:, 0:64], rc[:]
                )

            def epi_out(chunk):
                ci = chunk % 2
                nc.sync.dma_start(
                    out_v[:, 8 * ci:8 * ci + 8, :], outsb[:, 8 * ci:8 * ci + 8, :]
                )

            # ---- schedule (chunk-serial, software-pipelined) ----
            def diag_desc(chunk, m):
                base_kb = 0 if chunk % 2 == 0 else 8
                if m == 0:
                    return dict(chunk=chunk, kb=base_kb, c0=0, first=True,
                                mask=gmask[:, 0:1024])
                elif m < 7:
                    c0 = 128 * m
                    return dict(chunk=chunk, kb=base_kb + m, c0=c0,
                                mask=gmask[:, 0:1024 - c0])
                else:
                    return dict(chunk=chunk, kb=base_kb + 7, c0=768, mask=bmask)

            actions = []
            actions.append(("proj", 0))
            actions.append(("proj", 1))
            for m in range(8):
                actions.append(("slot", diag_desc(0, m)))
            actions.append(("proj", 4))
            for kb in range(16, 20):
                actions.append(("slot", dict(chunk=0, kb=kb)))
            actions.append(("proj", 5))
            for kb in range(20, 24):
                actions.append(("slot", dict(chunk=0, kb=kb, last=(kb == 23))))
            actions.append(("proj", 2))
            actions.append(("proj", 3))
            actions.append(("epi0", 0))        # copy accum -> SBUF
            for m in range(8):
                actions.append(("slot", diag_desc(1, m)))
                actions.append(("epi1", (0, m)))   # interleaved lo epilogue
            actions.append(("epi2", 0))        # lo output DMA
            for kb in range(0, 8):
                actions.append(("slot", dict(chunk=1, kb=kb)))
            for kb in range(16, 24):
                actions.append(("slot", dict(chunk=1, kb=kb)))
            actions.append(("proj", 6))
            for kb in range(24, 28):
                actions.append(("slot", dict(chunk=1, kb=kb)))
            actions.append(("proj", 7))
            for kb in range(28, 32):
                actions.append(("slot", dict(chunk=1, kb=kb, last=(kb == 31))))
            actions.append(("epi0", 1))
            for m in range(8):
                actions.append(("epi1", (1, m)))
            actions.append(("epi2", 1))

            pending = None  # (desc, pt) awaiting its PV half
            osbs = [None, None]

            def flush():
                nonlocal pending
                if pending is not None:
                    desc, pt = pending
                    pv(desc["chunk"], desc["kb"], pt,
                       c0=desc.get("c0", 0), first=desc.get("first", False),
                       last=desc.get("last", False))
                    pending = None

            for kind, arg in actions:
                if kind == "proj":
                    proj_panel(arg)
                elif kind == "slot":
                    pt = scores_exp(arg["chunk"], arg["kb"],
                                    c0=arg.get("c0", 0), mask=arg.get("mask"))
                    flush()
                    pending = (arg, pt)
                elif kind == "epi0":
                    flush()
                    osbs[arg % 2] = epi_start(arg)
                elif kind == "epi1":
                    c, m = arg
                    epi_piece(c, osbs[c % 2], m)
                else:  # epi2
                    epi_out(arg)
            flush()

    nc.compile()
    return nc


def make_inputs(x, Wk, Wv):
    """Build the 8 per-core input maps (pure layout work)."""
    bf16 = ml_dtypes.bfloat16
    wkv = np.concatenate([Wk, Wv], axis=1)            # [1024, 128]
    wkv_t = np.ascontiguousarray(
        wkv.reshape(NCHI, 128, 128).transpose(1, 0, 2)
    ).astype(bf16)  # [cp, chi, m]

    g = np.zeros((128, 1280), dtype=np.float32)
    jj = np.arange(1024)
    pp = np.arange(128)[:, None]
    g[:, 0:1024] = (jj[None, :] >= pp).astype(np.float32)
    g[:, 0:128] = np.triu(np.ones((128, 128), np.float32))
    jb = np.arange(256)
    g[:, 1024:1280] = (jb[None, :] >= (pp + 128)).astype(np.float32)
    g = g.astype(bf16)

    eye = np.zeros((128, 129), dtype=np.float32)
    eye[64:128, 0:64] = np.eye(64)
    eye[0:65, 64:129] = np.eye(65)

    in_maps = []
    for c in range(8):
        b, role = divmod(c, 2)
        lo_g, hi_g = ROLE_CHUNKS[role]
        others = sorted(set(range(4)) - {lo_g, hi_g})
        pan = [2 * lo_g, 2 * lo_g + 1, 2 * hi_g, 2 * hi_g + 1]
        for o in others:
            pan += [2 * o, 2 * o + 1]

        xT = np.ascontiguousarray(x[b].T)             # [1024, 4096]
        xr = xT.reshape(NCHI, 128, T)                 # [chi, cp, t]
        xt = np.empty((NPAN, 128, NCHI, PAN), dtype=bf16)
        for j, pg in enumerate(pan):
            xt[j] = xr[:, :, pg * PAN:(pg + 1) * PAN].transpose(1, 0, 2)

        gstart = np.empty(NKB, dtype=np.int64)        # global row of each kb
        for kb in range(NKB):
            gstart[kb] = pan[kb // 4] * PAN + (kb % 4) * 128
        vmask = np.zeros((128, 2 * NKB), dtype=np.float32)
        vmask[:, 0:NKB] = (gstart < (lo_g + 1) * CHUNK).astype(np.float32)[None, :]
        vmask[:, NKB:] = (gstart < (hi_g + 1) * CHUNK).astype(np.float32)[None, :]

        in_maps.append(
            {"xt": xt, "wkv": wkv_t, "vm": vmask, "mk": g, "eye": eye}
        )
    return in_maps


_NC = None


def get_nc():
    global _NC
    if _NC is None:
        _NC = build_nc()
    return _NC


def kernel(x, Wk, Wv):
    x = np.asarray(x, dtype=np.float32)
    Wk = np.asarray(Wk, dtype=np.float32)
    Wv = np.asarray(Wv, dtype=np.float32)
    nc = get_nc()
    in_maps = make_inputs(x, Wk, Wv)
    res = run_bass_kernel_spmd(nc, in_maps, list(range(8)))
    out = np.empty((B, T, H), dtype=np.float32)
    for c in range(8):
        b, role = divmod(c, 2)
        lo_g, hi_g = ROLE_CHUNKS[role]
        o = res.results[c]["out"]
        out[b, lo_g * CHUNK:(lo_g + 1) * CHUNK] = o[0:CHUNK]
        out[b, hi_g * CHUNK:(hi_g + 1) * CHUNK] = o[CHUNK:]
    return out
